# revision 1
# baseline (speedup 1.0000x reference)
"""Distributed 2-layer GCN (EADGNN, N=50000 E=800000 D=128) on 8 TRN2
NeuronCores via Bass/Tile.

Reference math (per layer l):
    h  = relu(A @ x @ W1[l] + b1[l])
    x' = A @ (h @ W2[l]) + b2[l]
with A = D^-1/2 (Adj + I) D^-1/2 (PyG gcn_norm, self-loops added).

Kernel strategy:
  * Propagation commutes with the dense matmuls: A @ (x W) == (A x) W, so all
    gather/scatter happens at width D=128 instead of 4D=512.
  * A is factored: gather tables store x~ = dinv * x (source-side scale), the
    scatter is a pure 0/1 one-hot matmul, and the target-side dinv is applied
    in the epilogue.  For the first half of a layer the target scale is
    commuted through the (bias-free, b1==0) relu:
        h = relu((dinv*raw) W1) = dinv * relu(raw W1)
    so the next table is t~ = dinv * (h W2) = dinv^2 * (relu(raw W1) W2).
  * Nodes are assigned to (core, tile-of-128, slot) positions by a balanced
    packer; each core owns TPC=49 tiles of 128 target slots.  Edges are
    partitioned by target tile, split by source half (dma_gather indices are
    int16, so tables are gathered as two <=25088-row halves), padded to
    CA/CB = 9/9 chunks of 128 edges per tile.
  * Per 128-edge chunk: dma_gather fetched the 128 source rows (f16) earlier
    in bulk, a one-hot S[e, t] = (iota == off_e) is built on the vector
    engine, and a PE matmul scatter-adds into PSUM (f32 accumulation).
    Self-loops are a contiguous DMA + identity matmul from the core's own
    local slice of the previous stage (no indices needed).
  * Between the four propagate stages the per-core slices are AllGathered
    into replicated tables (3 collectives; the final stage output stays
    local and the host undoes the node permutation).
"""
import os
import sys

sys.path.insert(0, "/opt/trn_rl_repo")
# A previously crashed session can leave cores wedged; always reset at init.
os.environ.setdefault("NEURON_RT_RESET_CORES", "1")

import numpy as np

from concourse import bacc, mybir, tile
from concourse import bass_utils
from concourse.masks import make_identity

P = 128

REAL_CFG = dict(N=50000, D=128, L=2, NCORES=8, TPC=49, CA=9, CB=9, GBLK=7)


def derived(cfg):
    d = dict(cfg)
    d["TGT"] = cfg["TPC"] * P                 # targets per core
    d["NPAD"] = cfg["NCORES"] * d["TGT"]      # padded node count
    d["HALF"] = d["NPAD"] // 2                # rows per gather table half
    assert d["HALF"] <= 32768                 # dma_gather int16 index limit
    assert cfg["TPC"] % cfg["GBLK"] == 0
    return d


# ----------------------------------------------------------------------------
# host-side graph preprocessing
# ----------------------------------------------------------------------------

def preprocess(edge_index, cfg, seed=0):
    """Assign nodes to (core, tile, slot) positions and build the per-core
    gather streams (wrapped int16 indices + per-chunk target offsets)."""
    c = derived(cfg)
    N, TPC, CA, CB, NC = c["N"], c["TPC"], c["CA"], c["CB"], c["NCORES"]
    TGT, HALF = c["TGT"], c["HALF"]
    row = np.asarray(edge_index[0], np.int64)
    col = np.asarray(edge_index[1], np.int64)

    deg = np.bincount(col, minlength=N).astype(np.float64) + 1.0  # + self loop
    dinv = (1.0 / np.sqrt(deg)).astype(np.float32)

    rng = np.random.default_rng(seed)
    # Split nodes into half A (cores 0..NC/2-1) and half B, balancing
    # out-degree sums (a node's half decides which gather table its
    # out-edges hit).
    outdeg = np.bincount(row, minlength=N)
    order = np.argsort(-outdeg, kind="stable")
    halfmark = np.zeros(N, bool)
    halfmark[order[::2]] = True   # True -> half A
    assert halfmark.sum() <= HALF and (N - halfmark.sum()) <= HALF

    a_edge = halfmark[row]
    a_in = np.bincount(col[a_edge], minlength=N)
    b_in = np.bincount(col[~a_edge], minlength=N)

    ntiles_half = (NC // 2) * TPC
    capA, capB = CA * P, CB * P

    def pack_half(nodes):
        """nodes -> grid [ntiles_half, P] of node ids (-1 pad) such that each
        tile's (A-edge, B-edge) loads fit the chunk capacities."""
        nn = len(nodes)
        slots = ntiles_half * P
        assert nn <= slots
        for _attempt in range(60):
            perm = rng.permutation(nn)
            grid = np.full(slots, -1, np.int64)
            grid[:nn] = nodes[perm]
            grid = grid.reshape(ntiles_half, P)
            av = np.where(grid >= 0, a_in[np.maximum(grid, 0)], 0)
            bv = np.where(grid >= 0, b_in[np.maximum(grid, 0)], 0)
            a_load, b_load = av.sum(1), bv.sum(1)
            for _ in range(3000):   # greedy repair by swapping heavy nodes
                badA, badB = a_load > capA, b_load > capB
                if not (badA.any() or badB.any()):
                    return grid
                if badA.any():
                    t_over, vals, loads = int(np.argmax(a_load)), av, a_load
                else:
                    t_over, vals, loads = int(np.argmax(b_load)), bv, b_load
                t_under = int(np.argmin(loads))
                s_over = int(np.argmax(vals[t_over]))
                s_under = int(np.argmin(vals[t_under]))
                n1, n2 = grid[t_over, s_over], grid[t_under, s_under]
                grid[t_over, s_over], grid[t_under, s_under] = n2, n1
                for arr, src in ((av, a_in), (bv, b_in)):
                    v1 = src[n1] if n1 >= 0 else 0
                    v2 = src[n2] if n2 >= 0 else 0
                    arr[t_over, s_over], arr[t_under, s_under] = v2, v1
                a_load = av.sum(1)
                b_load = bv.sum(1)
        raise RuntimeError("packing failed")

    gridA = pack_half(np.flatnonzero(halfmark))
    gridB = pack_half(np.flatnonzero(~halfmark))
    grid = np.concatenate([gridA, gridB], 0).reshape(NC, TPC, P)

    pos = np.full(N, -1, np.int64)
    flat = grid.reshape(-1)
    valid = flat >= 0
    pos[flat[valid]] = np.flatnonzero(valid)
    assert (pos >= 0).all()

    spos, tpos = pos[row], pos[col]
    tcore = tpos // TGT
    tblk = (tpos % TGT) // P
    toff = tpos % P
    is_a = spos < HALF

    idx_w, off_arr = {}, {}
    for half, CX in (("A", CA), ("B", CB)):
        sel = is_a if half == "A" else ~is_a
        sp = spos[sel] - (0 if half == "A" else HALF)
        key = tcore[sel] * TPC + tblk[sel]
        o = np.argsort(key, kind="stable")
        key_s, sp_s, to_s = key[o], sp[o], toff[sel][o]
        nblocks = NC * TPC
        cnts = np.bincount(key_s, minlength=nblocks)
        starts = np.concatenate([[0], np.cumsum(cnts)[:-1]])
        rank = np.arange(len(key_s)) - starts[key_s]
        assert rank.max(initial=0) < CX * P
        idx_full = np.zeros((NC, TPC, CX * P), np.int64)
        off_full = np.full((NC, TPC, CX * P), -1.0, np.float32)
        ci, bi = key_s // TPC, key_s % TPC
        idx_full[ci, bi, rank] = sp_s
        off_full[ci, bi, rank] = to_s
        # idx stream: flatten (blk, chunk, e) then wrap 16-way per dma_gather
        flat_i = idx_full.reshape(NC, TPC * CX * P)
        w = flat_i.reshape(NC, -1, 16).transpose(0, 2, 1).astype(np.int16)
        idx_w[half] = np.tile(w, (1, P // 16, 1))           # [NC, 128, cols]
        off_arr[half] = off_full.reshape(NC, TPC * CX, P).transpose(0, 2, 1).copy()

    dl = np.where(grid >= 0, dinv[np.maximum(grid, 0)], 0.0)  # [NC, TPC, P]
    dl = dl.transpose(0, 2, 1).astype(np.float32).copy()      # [NC, 128, TPC]

    return dict(pos=pos, dinv=dinv,
                idxA=idx_w["A"], idxB=idx_w["B"],
                offA=off_arr["A"], offB=off_arr["B"],
                dloc=dl, d2loc=(dl * dl).copy())


# ----------------------------------------------------------------------------
# bass kernel
# ----------------------------------------------------------------------------

def build_nc(cfg):
    c = derived(cfg)
    D, L, NC, TPC, CA, CB, GBLK = (c["D"], c["L"], c["NCORES"], c["TPC"],
                                   c["CA"], c["CB"], c["GBLK"])
    TGT, NPAD, HALF = c["TGT"], c["NPAD"], c["HALF"]
    f16, f32 = mybir.dt.float16, mybir.dt.float32
    i16, i32 = mybir.dt.int16, mybir.dt.int32

    nc = bacc.Bacc("TRN2", target_bir_lowering=False, debug=False,
                   num_devices=NC)

    def inp(name, shape, dt):
        return nc.dram_tensor(name, list(shape), dt, kind="ExternalInput").ap()

    xt = inp("xt", (NPAD, D), f16)
    xself = inp("xself", (TGT, D), f16)
    idxA = inp("idxA", (P, TPC * CA * 8), i16)
    idxB = inp("idxB", (P, TPC * CB * 8), i16)
    offA = inp("offA", (P, TPC * CA), f32)
    offB = inp("offB", (P, TPC * CB), f32)
    w1 = inp("w1", (L, D, 4 * D), f16)
    w2 = inp("w2", (L, 4 * D, D), f16)
    b1c = inp("b1c", (L, 4, D), f32)
    b2r = inp("b2r", (L, P, D), f32)
    dloc = inp("dloc", (P, TPC), f32)
    d2loc = inp("d2loc", (P, TPC), f32)
    y = nc.dram_tensor("y", [TGT, D], f32, kind="ExternalOutput").ap()

    rg = [list(range(NC))]

    with tile.TileContext(nc) as tc:
        with (
            tc.tile_pool(name="dram", bufs=1, space="DRAM") as dram,
            tc.tile_pool(name="const", bufs=1) as cp,
            tc.tile_pool(name="work", bufs=1) as wp,
            tc.tile_pool(name="psum", bufs=1, space="PSUM") as pp,
        ):
            t_loc = dram.tile([TGT, D], f16, name="t_loc")
            x1_loc = dram.tile([TGT, D], f16, name="x1_loc")
            t2_loc = dram.tile([TGT, D], f16, name="t2_loc")
            t_tab = dram.tile([NPAD, D], f16, name="t_tab", addr_space="Shared")
            x1_tab = dram.tile([NPAD, D], f16, name="x1_tab", addr_space="Shared")
            t2_tab = dram.tile([NPAD, D], f16, name="t2_tab", addr_space="Shared")

            iota_i = cp.tile([P, P], i32, name="iota_i")
            nc.gpsimd.iota(iota_i[:], pattern=[[1, P]], base=0, channel_multiplier=0)
            iota_f = cp.tile([P, P], f16, name="iota_f")
            nc.vector.tensor_copy(out=iota_f[:], in_=iota_i[:])
            ident = cp.tile([P, P], f16, name="ident")
            make_identity(nc, ident[:])

            w1_sb = cp.tile([P, L * 4 * D], f16, name="w1_sb")
            for l in range(L):
                nc.sync.dma_start(out=w1_sb[:, l * 4 * D:(l + 1) * 4 * D], in_=w1[l])
            w2_sb, b1_sb, b2_sb = [], [], []
            for l in range(L):
                w2_sb.append([])
                b1_sb.append([])
                for ci in range(4):
                    t = cp.tile([P, D], f16, name=f"w2_sb_{l}_{ci}")
                    nc.sync.dma_start(out=t[:], in_=w2[l, ci * P:(ci + 1) * P, :])
                    w2_sb[l].append(t)
                    t = cp.tile([P, 1], f32, name=f"b1_sb_{l}_{ci}")
                    nc.sync.dma_start(out=t[:], in_=b1c[l, ci, :, None])
                    b1_sb[l].append(t)
                t = cp.tile([P, D], f32, name=f"b2_sb_{l}")
                nc.sync.dma_start(out=t[:], in_=b2r[l])
                b2_sb.append(t)
            dl_sb = cp.tile([P, TPC], f32, name="dl_sb")
            nc.sync.dma_start(out=dl_sb[:], in_=dloc[:])
            d2_sb = cp.tile([P, TPC], f32, name="d2_sb")
            nc.sync.dma_start(out=d2_sb[:], in_=d2loc[:])

            idxA_sb = cp.tile([P, TPC * CA * 8], i16, name="idxA_sb")
            nc.sync.dma_start(out=idxA_sb[:], in_=idxA[:])
            idxB_sb = cp.tile([P, TPC * CB * 8], i16, name="idxB_sb")
            nc.sync.dma_start(out=idxB_sb[:], in_=idxB[:])
            offA_sb = cp.tile([P, TPC * CA], f32, name="offA_sb")
            nc.sync.dma_start(out=offA_sb[:], in_=offA[:])
            offB_sb = cp.tile([P, TPC * CB], f32, name="offB_sb")
            nc.sync.dma_start(out=offB_sb[:], in_=offB[:])

            def stage(l, kind, table_ap, selfsrc_ap, out_loc_ap, final=False):
                """kind 'p1': propagate (transposed acc [feat, tgt]) + dense
                mms -> t~ slice.  kind 'p2': propagate (natural acc
                [tgt, feat]) + dinv/bias epilogue."""
                tabA = table_ap[0:HALF, :]
                tabB = table_ap[HALF:NPAD, :]
                for g in range(TPC // GBLK):
                    gatA = wp.tile([P, GBLK * CA, D], f16, tag="gatA", bufs=2,
                                   name=f"gatA_{kind}{l}_{g}")
                    nc.gpsimd.dma_gather(
                        out_ap=gatA[:], in_ap=tabA,
                        idxs_ap=idxA_sb[:, g * GBLK * CA * 8:(g + 1) * GBLK * CA * 8],
                        num_idxs=GBLK * CA * P, num_idxs_reg=GBLK * CA * P,
                        elem_size=D, single_packet=False)
                    gatB = wp.tile([P, GBLK * CB, D], f16, tag="gatB", bufs=2,
                                   name=f"gatB_{kind}{l}_{g}")
                    nc.gpsimd.dma_gather(
                        out_ap=gatB[:], in_ap=tabB,
                        idxs_ap=idxB_sb[:, g * GBLK * CB * 8:(g + 1) * GBLK * CB * 8],
                        num_idxs=GBLK * CB * P, num_idxs_reg=GBLK * CB * P,
                        elem_size=D, single_packet=False)
                    for bb in range(GBLK):
                        b = g * GBLK + bb
                        selfT = wp.tile([P, D], f16, tag="selfT", bufs=3,
                                        name=f"selfT_{kind}{l}_{b}")
                        nc.sync.dma_start(out=selfT[:],
                                          in_=selfsrc_ap[b * P:(b + 1) * P, :])
                        acc = pp.tile([P, D], f32, tag="acc", bufs=2,
                                      name=f"acc_{kind}{l}_{b}", space="PSUM")
                        if kind == "p1":
                            nc.tensor.matmul(acc[:], lhsT=selfT[:], rhs=ident[:],
                                             start=True, stop=False)
                        else:
                            nc.tensor.matmul(acc[:], lhsT=ident[:], rhs=selfT[:],
                                             start=True, stop=False)
                        nchunks = CA + CB
                        for j in range(nchunks):
                            if j < CA:
                                m_ap = gatA[:, bb * CA + j, :]
                                off_ap = offA_sb[:, b * CA + j:b * CA + j + 1]
                            else:
                                jj = j - CA
                                m_ap = gatB[:, bb * CB + jj, :]
                                off_ap = offB_sb[:, b * CB + jj:b * CB + jj + 1]
                            s_t = wp.tile([P, P], f16, tag="s_t", bufs=4,
                                          name=f"s_{kind}{l}_{b}_{j}")
                            nc.vector.tensor_scalar(
                                out=s_t[:], in0=iota_f[:], scalar1=off_ap,
                                scalar2=None, op0=mybir.AluOpType.is_equal)
                            last = j == nchunks - 1
                            if kind == "p1":
                                nc.tensor.matmul(acc[:], lhsT=m_ap, rhs=s_t[:],
                                                 start=False, stop=last)
                            else:
                                nc.tensor.matmul(acc[:], lhsT=s_t[:], rhs=m_ap,
                                                 start=False, stop=last)
                        if kind == "p1":
                            p1t = wp.tile([P, P], f16, tag="p1t", bufs=2,
                                          name=f"p1t_{l}_{b}")
                            nc.scalar.activation(
                                out=p1t[:], in_=acc[:],
                                func=mybir.ActivationFunctionType.Copy,
                                bias=0.0, scale=1.0)
                            tps = pp.tile([P, D], f32, tag="tps", bufs=2,
                                          name=f"tps_{l}_{b}", space="PSUM")
                            for ci in range(4):
                                hps = pp.tile([P, P], f32, tag="hps", bufs=2,
                                              name=f"hps_{l}_{b}_{ci}", space="PSUM")
                                nc.tensor.matmul(
                                    hps[:],
                                    lhsT=w1_sb[:, (l * 4 + ci) * P:(l * 4 + ci + 1) * P],
                                    rhs=p1t[:], start=True, stop=True)
                                hT = wp.tile([P, P], f16, tag="hT", bufs=8,
                                             name=f"hT_{l}_{b}_{ci}")
                                nc.scalar.activation(
                                    out=hT[:], in_=hps[:],
                                    func=mybir.ActivationFunctionType.Relu,
                                    bias=b1_sb[l][ci][:, 0:1], scale=1.0)
                                nc.tensor.matmul(tps[:], lhsT=hT[:],
                                                 rhs=w2_sb[l][ci][:],
                                                 start=(ci == 0), stop=(ci == 3))
                            tsb = wp.tile([P, D], f16, tag="tsb", bufs=3,
                                          name=f"tsb_{l}_{b}")
                            nc.vector.tensor_scalar(
                                out=tsb[:], in0=tps[:],
                                scalar1=d2_sb[:, b:b + 1], scalar2=None,
                                op0=mybir.AluOpType.mult)
                            nc.sync.dma_start(out=out_loc_ap[b * P:(b + 1) * P, :],
                                              in_=tsb[:])
                        else:
                            tmp = wp.tile([P, D], f32, tag="ep_tmp", bufs=2,
                                          name=f"ept_{l}_{b}")
                            nc.vector.tensor_scalar(
                                out=tmp[:], in0=acc[:],
                                scalar1=dl_sb[:, b:b + 1], scalar2=None,
                                op0=mybir.AluOpType.mult)
                            if final:
                                osb = wp.tile([P, D], f32, tag="osb", bufs=3,
                                              name=f"osb_{l}_{b}")
                                nc.vector.tensor_tensor(
                                    out=osb[:], in0=tmp[:], in1=b2_sb[l][:],
                                    op=mybir.AluOpType.add)
                                nc.sync.dma_start(
                                    out=out_loc_ap[b * P:(b + 1) * P, :], in_=osb[:])
                            else:
                                tmp2 = wp.tile([P, D], f32, tag="ep_tmp2", bufs=2,
                                               name=f"ept2_{l}_{b}")
                                nc.vector.tensor_tensor(
                                    out=tmp2[:], in0=tmp[:], in1=b2_sb[l][:],
                                    op=mybir.AluOpType.add)
                                xsb = wp.tile([P, D], f16, tag="xsb", bufs=3,
                                              name=f"xsb_{l}_{b}")
                                nc.vector.tensor_scalar(
                                    out=xsb[:], in0=tmp2[:],
                                    scalar1=dl_sb[:, b:b + 1], scalar2=None,
                                    op0=mybir.AluOpType.mult)
                                nc.sync.dma_start(
                                    out=out_loc_ap[b * P:(b + 1) * P, :], in_=xsb[:])

            def ag(loc, tab):
                nc.gpsimd.collective_compute(
                    "AllGather", mybir.AluOpType.bypass, replica_groups=rg,
                    ins=[loc.opt()], outs=[tab.opt()])

            stage(0, "p1", xt, xself, t_loc[:])
            ag(t_loc, t_tab)
            stage(0, "p2", t_tab[:], t_loc[:], x1_loc[:])
            ag(x1_loc, x1_tab)
            stage(1, "p1", x1_tab[:], x1_loc[:], t2_loc[:])
            ag(t2_loc, t2_tab)
            stage(1, "p2", t2_tab[:], t2_loc[:], y, final=True)

    nc.compile()
    return nc


# ----------------------------------------------------------------------------
# host glue
# ----------------------------------------------------------------------------

def make_in_maps(inputs, prep, cfg):
    c = derived(cfg)
    D, L, NC = c["D"], c["L"], c["NCORES"]
    TGT, NPAD = c["TGT"], c["NPAD"]
    x = np.asarray(inputs["x"], np.float32)
    W1 = np.asarray(inputs["W1"], np.float32)
    W2 = np.asarray(inputs["W2"], np.float32)
    b1 = np.asarray(inputs["b1"], np.float32)
    b2 = np.asarray(inputs["b2"], np.float32)

    pos, dinv = prep["pos"], prep["dinv"]
    xt = np.zeros((NPAD, D), np.float16)
    xt[pos] = (x * dinv[:, None]).astype(np.float16)

    w1f = W1.astype(np.float16)
    w2f = W2.astype(np.float16)
    b1c = b1.reshape(L, 4, D).astype(np.float32)
    b2r = np.broadcast_to(b2[:, None, :], (L, P, D)).astype(np.float32).copy()

    in_maps = []
    for m in range(NC):
        in_maps.append(dict(
            xt=xt, xself=xt[m * TGT:(m + 1) * TGT].copy(),
            idxA=prep["idxA"][m], idxB=prep["idxB"][m],
            offA=prep["offA"][m], offB=prep["offB"][m],
            w1=w1f, w2=w2f, b1c=b1c, b2r=b2r,
            dloc=prep["dloc"][m], d2loc=prep["d2loc"][m],
        ))
    return in_maps


def assemble_output(results, prep, cfg):
    c = derived(cfg)
    D, NC, TGT = c["D"], c["NCORES"], c["TGT"]
    full = np.empty((c["NPAD"], D), np.float32)
    for m in range(NC):
        full[m * TGT:(m + 1) * TGT] = results[m]["y"]
    return full[prep["pos"]]


_NC_CACHE = {}


def get_nc(cfg_key=None):
    key = "real"
    if key not in _NC_CACHE:
        _NC_CACHE[key] = build_nc(REAL_CFG)
    return _NC_CACHE[key]


def kernel(edge_index, x, W1, b1, W2, b2, ix=0):
    cfg = REAL_CFG
    edge_index = np.asarray(edge_index, np.int64)
    inputs = dict(x=np.asarray(x), W1=np.asarray(W1), b1=np.asarray(b1),
                  W2=np.asarray(W2), b2=np.asarray(b2))
    assert edge_index.shape[0] == 2
    assert inputs["x"].shape == (cfg["N"], cfg["D"])

    prep = preprocess(edge_index, cfg)
    in_maps = make_in_maps(inputs, prep, cfg)
    nc = get_nc()
    res = bass_utils.run_bass_kernel_spmd(
        nc, in_maps, core_ids=list(range(cfg["NCORES"])), trace=False)
    return assemble_output(res.results, prep, cfg)


# revision 3
# speedup vs baseline: 270.1797x; 270.1797x over previous
"""Distributed 2-layer GCN (EADGNN, N=50000 E=800000 D=128) on 8 TRN2
NeuronCores via Bass/Tile.

Reference math (per layer l):
    h  = relu(A @ x @ W1[l] + b1[l])
    x' = A @ (h @ W2[l]) + b2[l]
with A = D^-1/2 (Adj + I) D^-1/2 (PyG gcn_norm, self-loops added).

Kernel strategy:
  * Propagation commutes with the dense matmuls: A @ (x W) == (A x) W, so all
    gather/scatter happens at width D=128 instead of 4D=512.
  * A is factored: gather tables store x~ = dinv * x (source-side scale), the
    scatter is a pure 0/1 one-hot matmul, and the target-side dinv is applied
    in the epilogue.  For the first half of a layer the target scale is
    commuted through the (bias-free, b1==0) relu:
        h = relu((dinv*raw) W1) = dinv * relu(raw W1)
    so the next table is t~ = dinv * (h W2) = dinv^2 * (relu(raw W1) W2).
  * Nodes are assigned to (core, tile-of-128, slot) positions by a balanced
    packer; each core owns TPC=49 tiles of 128 target slots.  Edges are
    partitioned by target tile, split by source half (dma_gather indices are
    int16, so tables are gathered as two <=25088-row halves), padded to
    CA/CB = 9/9 chunks of 128 edges per tile.
  * Per 128-edge chunk: dma_gather fetched the 128 source rows (f16) earlier
    in bulk, a one-hot S[e, t] = (iota == off_e) is built on the vector
    engine, and a PE matmul scatter-adds into PSUM (f32 accumulation).
    Self-loops are a contiguous DMA + identity matmul from the core's own
    local slice of the previous stage (no indices needed).
  * Between the four propagate stages the per-core slices are AllGathered
    into replicated tables (3 collectives; the final stage output stays
    local and the host undoes the node permutation).
"""
import os
import sys

sys.path.insert(0, "/opt/trn_rl_repo")
# A previously crashed session can leave cores wedged; always reset at init.
os.environ.setdefault("NEURON_RT_RESET_CORES", "1")

import numpy as np

from concourse import bacc, mybir, tile
from concourse import bass_utils
from concourse.masks import make_identity

P = 128

REAL_CFG = dict(N=50000, D=128, L=2, NCORES=8, TPC=49, CA=9, CB=9, GBLK=7)


def derived(cfg):
    d = dict(cfg)
    d["TGT"] = cfg["TPC"] * P                 # targets per core
    d["NPAD"] = cfg["NCORES"] * d["TGT"]      # padded node count
    d["HALF"] = d["NPAD"] // 2                # rows per gather table half
    assert d["HALF"] <= 32768                 # dma_gather int16 index limit
    assert cfg["TPC"] % cfg["GBLK"] == 0
    return d


# ----------------------------------------------------------------------------
# host-side graph preprocessing
# ----------------------------------------------------------------------------

def preprocess(edge_index, cfg, seed=0):
    """Assign nodes to (core, tile, slot) positions and build the per-core
    gather streams (wrapped int16 indices + per-chunk target offsets)."""
    c = derived(cfg)
    N, TPC, CA, CB, NC = c["N"], c["TPC"], c["CA"], c["CB"], c["NCORES"]
    TGT, HALF = c["TGT"], c["HALF"]
    row = np.asarray(edge_index[0], np.int64)
    col = np.asarray(edge_index[1], np.int64)

    deg = np.bincount(col, minlength=N).astype(np.float64) + 1.0  # + self loop
    dinv = (1.0 / np.sqrt(deg)).astype(np.float32)

    rng = np.random.default_rng(seed)
    # Split nodes into half A (cores 0..NC/2-1) and half B, balancing
    # out-degree sums (a node's half decides which gather table its
    # out-edges hit).
    outdeg = np.bincount(row, minlength=N)
    order = np.argsort(-outdeg, kind="stable")
    halfmark = np.zeros(N, bool)
    halfmark[order[::2]] = True   # True -> half A
    assert halfmark.sum() <= HALF and (N - halfmark.sum()) <= HALF

    a_edge = halfmark[row]
    a_in = np.bincount(col[a_edge], minlength=N)
    b_in = np.bincount(col[~a_edge], minlength=N)

    ntiles_half = (NC // 2) * TPC
    capA, capB = CA * P, CB * P

    def pack_half(nodes):
        """nodes -> grid [ntiles_half, P] of node ids (-1 pad) such that each
        tile's (A-edge, B-edge) loads fit the chunk capacities."""
        nn = len(nodes)
        slots = ntiles_half * P
        assert nn <= slots
        for _attempt in range(60):
            perm = rng.permutation(nn)
            grid = np.full(slots, -1, np.int64)
            grid[:nn] = nodes[perm]
            grid = grid.reshape(ntiles_half, P)
            av = np.where(grid >= 0, a_in[np.maximum(grid, 0)], 0)
            bv = np.where(grid >= 0, b_in[np.maximum(grid, 0)], 0)
            a_load, b_load = av.sum(1), bv.sum(1)
            for _ in range(3000):   # greedy repair by swapping heavy nodes
                badA, badB = a_load > capA, b_load > capB
                if not (badA.any() or badB.any()):
                    return grid
                if badA.any():
                    t_over, vals, loads = int(np.argmax(a_load)), av, a_load
                else:
                    t_over, vals, loads = int(np.argmax(b_load)), bv, b_load
                t_under = int(np.argmin(loads))
                s_over = int(np.argmax(vals[t_over]))
                s_under = int(np.argmin(vals[t_under]))
                n1, n2 = grid[t_over, s_over], grid[t_under, s_under]
                grid[t_over, s_over], grid[t_under, s_under] = n2, n1
                for arr, src in ((av, a_in), (bv, b_in)):
                    v1 = src[n1] if n1 >= 0 else 0
                    v2 = src[n2] if n2 >= 0 else 0
                    arr[t_over, s_over], arr[t_under, s_under] = v2, v1
                a_load = av.sum(1)
                b_load = bv.sum(1)
        raise RuntimeError("packing failed")

    gridA = pack_half(np.flatnonzero(halfmark))
    gridB = pack_half(np.flatnonzero(~halfmark))
    grid = np.concatenate([gridA, gridB], 0).reshape(NC, TPC, P)

    pos = np.full(N, -1, np.int64)
    flat = grid.reshape(-1)
    valid = flat >= 0
    pos[flat[valid]] = np.flatnonzero(valid)
    assert (pos >= 0).all()

    spos, tpos = pos[row], pos[col]
    tcore = tpos // TGT
    tblk = (tpos % TGT) // P
    toff = tpos % P
    is_a = spos < HALF

    idx_w, off_arr = {}, {}
    for half, CX in (("A", CA), ("B", CB)):
        sel = is_a if half == "A" else ~is_a
        sp = spos[sel] - (0 if half == "A" else HALF)
        key = tcore[sel] * TPC + tblk[sel]
        o = np.argsort(key, kind="stable")
        key_s, sp_s, to_s = key[o], sp[o], toff[sel][o]
        nblocks = NC * TPC
        cnts = np.bincount(key_s, minlength=nblocks)
        starts = np.concatenate([[0], np.cumsum(cnts)[:-1]])
        rank = np.arange(len(key_s)) - starts[key_s]
        assert rank.max(initial=0) < CX * P
        idx_full = np.zeros((NC, TPC, CX * P), np.int64)
        off_full = np.full((NC, TPC, CX * P), -1.0, np.float32)
        ci, bi = key_s // TPC, key_s % TPC
        idx_full[ci, bi, rank] = sp_s
        off_full[ci, bi, rank] = to_s
        # idx stream: flatten (blk, chunk, e) then wrap 16-way per dma_gather
        flat_i = idx_full.reshape(NC, TPC * CX * P)
        w = flat_i.reshape(NC, -1, 16).transpose(0, 2, 1).astype(np.int16)
        idx_w[half] = np.tile(w, (1, P // 16, 1))           # [NC, 128, cols]
        off_arr[half] = off_full.reshape(NC, TPC * CX, P).transpose(0, 2, 1).copy()

    dl = np.where(grid >= 0, dinv[np.maximum(grid, 0)], 0.0)  # [NC, TPC, P]
    dl = dl.transpose(0, 2, 1).astype(np.float32).copy()      # [NC, 128, TPC]

    return dict(pos=pos, dinv=dinv,
                idxA=idx_w["A"], idxB=idx_w["B"],
                offA=off_arr["A"], offB=off_arr["B"],
                dloc=dl, d2loc=(dl * dl).copy())


# ----------------------------------------------------------------------------
# bass kernel
# ----------------------------------------------------------------------------

def build_nc(cfg, repeat=1):
    c = derived(cfg)
    D, L, NC, TPC, CA, CB, GBLK = (c["D"], c["L"], c["NCORES"], c["TPC"],
                                   c["CA"], c["CB"], c["GBLK"])
    TGT, NPAD, HALF = c["TGT"], c["NPAD"], c["HALF"]
    f16, f32 = mybir.dt.float16, mybir.dt.float32
    i16, i32 = mybir.dt.int16, mybir.dt.int32

    nc = bacc.Bacc("TRN2", target_bir_lowering=False, debug=False,
                   num_devices=NC)

    def inp(name, shape, dt):
        return nc.dram_tensor(name, list(shape), dt, kind="ExternalInput").ap()

    xt = inp("xt", (NPAD, D), f16)
    xself = inp("xself", (TGT, D), f16)
    idxA = inp("idxA", (P, TPC * CA * 8), i16)
    idxB = inp("idxB", (P, TPC * CB * 8), i16)
    offA = inp("offA", (P, TPC * CA), f32)
    offB = inp("offB", (P, TPC * CB), f32)
    w1 = inp("w1", (L, D, 4 * D), f16)
    w2 = inp("w2", (L, 4 * D, D), f16)
    b1c = inp("b1c", (L, 4, D), f32)
    b2r = inp("b2r", (L, P, D), f32)
    dloc = inp("dloc", (P, TPC), f32)
    d2loc = inp("d2loc", (P, TPC), f32)
    y = nc.dram_tensor("y", [TGT, D], f32, kind="ExternalOutput").ap()

    rg = [list(range(NC))]

    with tile.TileContext(nc) as tc:
        with (
            tc.tile_pool(name="dram", bufs=1, space="DRAM") as dram,
            tc.tile_pool(name="const", bufs=1) as cp,
            tc.tile_pool(name="work", bufs=1) as wp,
            tc.tile_pool(name="psum", bufs=1, space="PSUM") as pp,
        ):

            iota_i = cp.tile([P, P], i32, name="iota_i")
            nc.gpsimd.iota(iota_i[:], pattern=[[1, P]], base=0, channel_multiplier=0)
            iota_f = cp.tile([P, P], f16, name="iota_f")
            nc.vector.tensor_copy(out=iota_f[:], in_=iota_i[:])
            ident = cp.tile([P, P], f16, name="ident")
            make_identity(nc, ident[:])

            w1_sb = cp.tile([P, L * 4 * D], f16, name="w1_sb")
            for l in range(L):
                nc.sync.dma_start(out=w1_sb[:, l * 4 * D:(l + 1) * 4 * D], in_=w1[l])
            w2_sb, b1_sb, b2_sb = [], [], []
            for l in range(L):
                w2_sb.append([])
                b1_sb.append([])
                for ci in range(4):
                    t = cp.tile([P, D], f16, name=f"w2_sb_{l}_{ci}")
                    nc.sync.dma_start(out=t[:], in_=w2[l, ci * P:(ci + 1) * P, :])
                    w2_sb[l].append(t)
                    t = cp.tile([P, 1], f32, name=f"b1_sb_{l}_{ci}")
                    nc.sync.dma_start(out=t[:], in_=b1c[l, ci, :, None])
                    b1_sb[l].append(t)
                t = cp.tile([P, D], f32, name=f"b2_sb_{l}")
                nc.sync.dma_start(out=t[:], in_=b2r[l])
                b2_sb.append(t)
            dl_sb = cp.tile([P, TPC], f32, name="dl_sb")
            nc.sync.dma_start(out=dl_sb[:], in_=dloc[:])
            d2_sb = cp.tile([P, TPC], f32, name="d2_sb")
            nc.sync.dma_start(out=d2_sb[:], in_=d2loc[:])

            idxA_sb = cp.tile([P, TPC * CA * 8], i16, name="idxA_sb")
            nc.sync.dma_start(out=idxA_sb[:], in_=idxA[:])
            idxB_sb = cp.tile([P, TPC * CB * 8], i16, name="idxB_sb")
            nc.sync.dma_start(out=idxB_sb[:], in_=idxB[:])
            offA_sb = cp.tile([P, TPC * CA], f32, name="offA_sb")
            nc.sync.dma_start(out=offA_sb[:], in_=offA[:])
            offB_sb = cp.tile([P, TPC * CB], f32, name="offB_sb")
            nc.sync.dma_start(out=offB_sb[:], in_=offB[:])

            rep_cell = [0]

            def stage(l, kind, table_ap, selfsrc_ap, out_loc_ap, final=False):
                rep_cell[0] += 1
                uniq = f"{kind}r{rep_cell[0]}"
                """kind 'p1': propagate (transposed acc [feat, tgt]) + dense
                mms -> t~ slice.  kind 'p2': propagate (natural acc
                [tgt, feat]) + dinv/bias epilogue."""
                tabA = table_ap[0:HALF, :]
                tabB = table_ap[HALF:NPAD, :]
                for g in range(TPC // GBLK):
                    gatA = wp.tile([P, GBLK * CA, D], f16, tag="gatA", bufs=2,
                                   name=f"gatA_{uniq}{l}_{g}")
                    nc.gpsimd.dma_gather(
                        out_ap=gatA[:], in_ap=tabA,
                        idxs_ap=idxA_sb[:, g * GBLK * CA * 8:(g + 1) * GBLK * CA * 8],
                        num_idxs=GBLK * CA * P, num_idxs_reg=GBLK * CA * P,
                        elem_size=D, single_packet=False)
                    gatB = wp.tile([P, GBLK * CB, D], f16, tag="gatB", bufs=2,
                                   name=f"gatB_{uniq}{l}_{g}")
                    nc.gpsimd.dma_gather(
                        out_ap=gatB[:], in_ap=tabB,
                        idxs_ap=idxB_sb[:, g * GBLK * CB * 8:(g + 1) * GBLK * CB * 8],
                        num_idxs=GBLK * CB * P, num_idxs_reg=GBLK * CB * P,
                        elem_size=D, single_packet=False)
                    for bb in range(GBLK):
                        b = g * GBLK + bb
                        selfT = wp.tile([P, D], f16, tag="selfT", bufs=3,
                                        name=f"selfT_{uniq}{l}_{b}")
                        nc.sync.dma_start(out=selfT[:],
                                          in_=selfsrc_ap[b * P:(b + 1) * P, :])
                        acc = pp.tile([P, D], f32, tag="acc", bufs=2,
                                      name=f"acc_{uniq}{l}_{b}", space="PSUM")
                        if kind == "p1":
                            nc.tensor.matmul(acc[:], lhsT=selfT[:], rhs=ident[:],
                                             start=True, stop=False)
                        else:
                            nc.tensor.matmul(acc[:], lhsT=ident[:], rhs=selfT[:],
                                             start=True, stop=False)
                        nchunks = CA + CB
                        for j in range(nchunks):
                            if j < CA:
                                m_ap = gatA[:, bb * CA + j, :]
                                off_ap = offA_sb[:, b * CA + j:b * CA + j + 1]
                            else:
                                jj = j - CA
                                m_ap = gatB[:, bb * CB + jj, :]
                                off_ap = offB_sb[:, b * CB + jj:b * CB + jj + 1]
                            s_t = wp.tile([P, P], f16, tag="s_t", bufs=4,
                                          name=f"s_{uniq}{l}_{b}_{j}")
                            nc.vector.tensor_scalar(
                                out=s_t[:], in0=iota_f[:], scalar1=off_ap,
                                scalar2=None, op0=mybir.AluOpType.is_equal)
                            last = j == nchunks - 1
                            if kind == "p1":
                                nc.tensor.matmul(acc[:], lhsT=m_ap, rhs=s_t[:],
                                                 start=False, stop=last)
                            else:
                                nc.tensor.matmul(acc[:], lhsT=s_t[:], rhs=m_ap,
                                                 start=False, stop=last)
                        if kind == "p1":
                            p1t = wp.tile([P, P], f16, tag="p1t", bufs=2,
                                          name=f"p1t_{uniq}{l}_{b}")
                            nc.scalar.activation(
                                out=p1t[:], in_=acc[:],
                                func=mybir.ActivationFunctionType.Copy,
                                bias=0.0, scale=1.0)
                            tps = pp.tile([P, D], f32, tag="tps", bufs=2,
                                          name=f"tps_{uniq}{l}_{b}", space="PSUM")
                            for ci in range(4):
                                hps = pp.tile([P, P], f32, tag="hps", bufs=2,
                                              name=f"hps_{uniq}{l}_{b}_{ci}", space="PSUM")
                                nc.tensor.matmul(
                                    hps[:],
                                    lhsT=w1_sb[:, (l * 4 + ci) * P:(l * 4 + ci + 1) * P],
                                    rhs=p1t[:], start=True, stop=True)
                                hT = wp.tile([P, P], f16, tag="hT", bufs=8,
                                             name=f"hT_{uniq}{l}_{b}_{ci}")
                                nc.scalar.activation(
                                    out=hT[:], in_=hps[:],
                                    func=mybir.ActivationFunctionType.Relu,
                                    bias=b1_sb[l][ci][:, 0:1], scale=1.0)
                                nc.tensor.matmul(tps[:], lhsT=hT[:],
                                                 rhs=w2_sb[l][ci][:],
                                                 start=(ci == 0), stop=(ci == 3))
                            tsb = wp.tile([P, D], f16, tag="tsb", bufs=3,
                                          name=f"tsb_{uniq}{l}_{b}")
                            nc.vector.tensor_scalar(
                                out=tsb[:], in0=tps[:],
                                scalar1=d2_sb[:, b:b + 1], scalar2=None,
                                op0=mybir.AluOpType.mult)
                            nc.sync.dma_start(out=out_loc_ap[b * P:(b + 1) * P, :],
                                              in_=tsb[:])
                        else:
                            tmp = wp.tile([P, D], f32, tag="ep_tmp", bufs=2,
                                          name=f"ept_{uniq}{l}_{b}")
                            nc.vector.tensor_scalar(
                                out=tmp[:], in0=acc[:],
                                scalar1=dl_sb[:, b:b + 1], scalar2=None,
                                op0=mybir.AluOpType.mult)
                            if final:
                                osb = wp.tile([P, D], f32, tag="osb", bufs=3,
                                              name=f"osb_{uniq}{l}_{b}")
                                nc.vector.tensor_tensor(
                                    out=osb[:], in0=tmp[:], in1=b2_sb[l][:],
                                    op=mybir.AluOpType.add)
                                nc.sync.dma_start(
                                    out=out_loc_ap[b * P:(b + 1) * P, :], in_=osb[:])
                            else:
                                tmp2 = wp.tile([P, D], f32, tag="ep_tmp2", bufs=2,
                                               name=f"ept2_{uniq}{l}_{b}")
                                nc.vector.tensor_tensor(
                                    out=tmp2[:], in0=tmp[:], in1=b2_sb[l][:],
                                    op=mybir.AluOpType.add)
                                xsb = wp.tile([P, D], f16, tag="xsb", bufs=3,
                                              name=f"xsb_{uniq}{l}_{b}")
                                nc.vector.tensor_scalar(
                                    out=xsb[:], in0=tmp2[:],
                                    scalar1=dl_sb[:, b:b + 1], scalar2=None,
                                    op0=mybir.AluOpType.mult)
                                nc.sync.dma_start(
                                    out=out_loc_ap[b * P:(b + 1) * P, :], in_=xsb[:])

            def ag(loc, tab):
                nc.gpsimd.collective_compute(
                    "AllGather", mybir.AluOpType.bypass, replica_groups=rg,
                    ins=[loc.opt()], outs=[tab.opt()])

            for _r in range(repeat):
                t_loc = dram.tile([TGT, D], f16, name=f"t_loc_{_r}")
                x1_loc = dram.tile([TGT, D], f16, name=f"x1_loc_{_r}")
                t2_loc = dram.tile([TGT, D], f16, name=f"t2_loc_{_r}")
                t_tab = dram.tile([NPAD, D], f16, name=f"t_tab_{_r}", addr_space="Shared")
                x1_tab = dram.tile([NPAD, D], f16, name=f"x1_tab_{_r}", addr_space="Shared")
                t2_tab = dram.tile([NPAD, D], f16, name=f"t2_tab_{_r}", addr_space="Shared")
                stage(0, "p1", xt, xself, t_loc[:])
                ag(t_loc, t_tab)
                stage(0, "p2", t_tab[:], t_loc[:], x1_loc[:])
                ag(x1_loc, x1_tab)
                stage(1, "p1", x1_tab[:], x1_loc[:], t2_loc[:])
                ag(t2_loc, t2_tab)
                stage(1, "p2", t2_tab[:], t2_loc[:], y, final=True)

    nc.compile()
    return nc


# ----------------------------------------------------------------------------
# host glue
# ----------------------------------------------------------------------------

def make_in_maps(inputs, prep, cfg):
    c = derived(cfg)
    D, L, NC = c["D"], c["L"], c["NCORES"]
    TGT, NPAD = c["TGT"], c["NPAD"]
    x = np.asarray(inputs["x"], np.float32)
    W1 = np.asarray(inputs["W1"], np.float32)
    W2 = np.asarray(inputs["W2"], np.float32)
    b1 = np.asarray(inputs["b1"], np.float32)
    b2 = np.asarray(inputs["b2"], np.float32)

    pos, dinv = prep["pos"], prep["dinv"]
    xt = np.zeros((NPAD, D), np.float16)
    xt[pos] = (x * dinv[:, None]).astype(np.float16)

    w1f = W1.astype(np.float16)
    w2f = W2.astype(np.float16)
    b1c = b1.reshape(L, 4, D).astype(np.float32)
    b2r = np.broadcast_to(b2[:, None, :], (L, P, D)).astype(np.float32).copy()

    in_maps = []
    for m in range(NC):
        in_maps.append(dict(
            xt=xt, xself=xt[m * TGT:(m + 1) * TGT].copy(),
            idxA=prep["idxA"][m], idxB=prep["idxB"][m],
            offA=prep["offA"][m], offB=prep["offB"][m],
            w1=w1f, w2=w2f, b1c=b1c, b2r=b2r,
            dloc=prep["dloc"][m], d2loc=prep["d2loc"][m],
        ))
    return in_maps


def assemble_output(results, prep, cfg):
    c = derived(cfg)
    D, NC, TGT = c["D"], c["NCORES"], c["TGT"]
    full = np.empty((c["NPAD"], D), np.float32)
    for m in range(NC):
        full[m * TGT:(m + 1) * TGT] = results[m]["y"]
    return full[prep["pos"]]


_NC_CACHE = {}


def get_nc(cfg_key=None):
    key = "real"
    if key not in _NC_CACHE:
        _NC_CACHE[key] = build_nc(REAL_CFG)
    return _NC_CACHE[key]


def kernel(edge_index, x, W1, b1, W2, b2, ix=0):
    cfg = REAL_CFG
    edge_index = np.asarray(edge_index, np.int64)
    inputs = dict(x=np.asarray(x), W1=np.asarray(W1), b1=np.asarray(b1),
                  W2=np.asarray(W2), b2=np.asarray(b2))
    assert edge_index.shape[0] == 2
    assert inputs["x"].shape == (cfg["N"], cfg["D"])

    prep = preprocess(edge_index, cfg)
    in_maps = make_in_maps(inputs, prep, cfg)
    nc = get_nc()
    res = bass_utils.run_bass_kernel_spmd(
        nc, in_maps, core_ids=list(range(cfg["NCORES"])), trace=False)
    return assemble_output(res.results, prep, cfg)
